# revision 34
# baseline (speedup 1.0000x reference)
"""Trainium2 Bass kernel for nn_MultiHeadAttention_45037027065972.

Head-parallel sharding: the reference's reshape `(B,S,H*D) -> (B,H,S,D)`
means head h of batch b only reads rows [128h, 128h+128) of the projection
inputs.  32 (b,h) slices are sharded 4-per-core across 8 cores (cores 0-3:
batch 0, cores 4-7: batch 1).  Each core projects its 4 slabs, runs full
S x S causal attention per slice in a transposed (k-major) layout, folds
the per-head output projection, and emits a per-core partial of
`sum_h out_h @ Wo_h` (shape [e=128, q=2048]).  The host unshard sums the
4 partials per batch, transposes, and adds bo.

Attention (S=2048, D=128), matmul operands bf16 (PSUM accumulate fp32):
  panel-outer (q panels of 512, descending), slice-inner loop; per slice
  scoresT[k,q] tiles = (K^T chunk stationary) @ (Q^T panel moving)
  P~ = exp(scoresT/sqrt(D)) on ACT (scores in [-9,9]: no running max);
  causal zeroing of the two diagonal chunks fused into one gpsimd
  affine_select over [128, 2, 512] (affine in (half, q, k)).
  oT[d,q]  += V-chunk @ P~        (PSUM accumulation over k chunks)
  lB[*,q]  += ones128 @ P~        (row-sum broadcast across partitions)
  rb = 1/lB via DVE reciprocal_approx_fast; osbn = oT * rb (bf16)
  acc[e,q] += Wo_h^T @ osbn       (PSUM accumulation across the 4 slices)
"""

import sys
import math
from collections import deque

import numpy as np

for _p in ("/opt/trn_rl_repo", "/opt/pypackages"):
    if _p not in sys.path:
        sys.path.append(_p)

import ml_dtypes
import concourse.bacc as bacc
import concourse.mybir as mybir
import concourse.tile as tile
from concourse.bass_utils import run_bass_kernel_spmd

B, S, H, D = 2, 2048, 16, 128
NCORES = 8
NSLICE = 4            # (b,h) slices per core
PANEL = 512           # q panel width
NPANEL = S // PANEL   # 4
SCALE = 1.0 / math.sqrt(128.0)
F32 = mybir.dt.float32
BF16 = mybir.dt.bfloat16
F32R = mybir.dt.float32r
AF = mybir.ActivationFunctionType
ALU = mybir.AluOpType
BF_NP = ml_dtypes.bfloat16

_CACHE = {}
_ONES = np.ones((128, 128), BF_NP)


def _build():
    nc = bacc.Bacc(trn_type="TRN2", target_bir_lowering=False, debug=False)

    qT_d = nc.dram_tensor("qT", [128, NSLICE * 128], BF16, kind="ExternalInput")
    kT_d = nc.dram_tensor("kT", [128, NSLICE * 128], BF16, kind="ExternalInput")
    vT_d = nc.dram_tensor("vT", [128, NSLICE * 128], BF16, kind="ExternalInput")
    Wq_d = nc.dram_tensor("Wq", [128, 2048], BF16, kind="ExternalInput")
    Wk_d = nc.dram_tensor("Wk", [128, 2048], BF16, kind="ExternalInput")
    Wv_d = nc.dram_tensor("Wv", [128, 2048], BF16, kind="ExternalInput")
    Wo4_d = nc.dram_tensor("Wo4", [128, NSLICE * 128], BF16, kind="ExternalInput")
    bqT_d = nc.dram_tensor("bqT", [128, 16], F32, kind="ExternalInput")
    bkT_d = nc.dram_tensor("bkT", [128, 16], F32, kind="ExternalInput")
    bv_d = nc.dram_tensor("bv_r", [1, 2048], BF16, kind="ExternalInput")
    ones_d = nc.dram_tensor("ones", [128, 128], BF16, kind="ExternalInput")
    out_d = nc.dram_tensor("partial", [128, S], BF16, kind="ExternalOutput")
    vscr_d = nc.dram_tensor("vscratch", [NSLICE, 128, 2048], BF16)

    with tile.TileContext(nc) as tc:
        with (
            tc.tile_pool(name="const", bufs=1) as const,
            tc.tile_pool(name="slab", bufs=1) as slab,
            tc.tile_pool(name="vslp", bufs=2) as vslp,
            tc.tile_pool(name="pbp", bufs=4) as pbp,
            tc.tile_pool(name="osbp", bufs=2) as osbp,
            tc.tile_pool(name="rbp", bufs=2) as rbp,
            tc.tile_pool(name="accp", bufs=2) as accp,
            tc.tile_pool(name="psS", bufs=2, space="PSUM") as psS,
            tc.tile_pool(name="psA", bufs=2, space="PSUM") as psA,
            tc.tile_pool(name="psO", bufs=2, space="PSUM") as psO,
        ):
            # ---- resident constants; DMA order = first-use order ----
            biasT = {}
            Wsb = {}
            xT = {}
            xdr = {"v": vT_d, "q": qT_d, "k": kT_d}
            wdr = {"v": Wv_d, "q": Wq_d, "k": Wk_d}
            for name in ("v", "q", "k"):
                t = const.tile([128, NSLICE * 128], BF16, tag=f"x{name}")
                xT[name] = t
                w = const.tile([128, 2048], BF16, tag=f"W{name}")
                Wsb[name] = w
            for name, dram in (("q", bqT_d), ("k", bkT_d)):
                t = const.tile([128, 16], F32, tag=f"bT{name}")
                biasT[name] = t
            ones_sb = const.tile([128, 128], BF16, tag="ones")
            bv_sb = const.tile([1, 2048], BF16, tag="bv")
            wo4 = const.tile([128, NSLICE * 128], BF16, tag="wo4")
            # Phase-1 DMAs: only what the Q/K projection reads (the
            # serial DVE eviction stream gates the attention start, so
            # the q/k pt matmuls must begin as early as possible).  The
            # V-projection DMAs are issued AFTER the Q/K projection is
            # emitted, so the pt matmuls' DMA-completion waits don't
            # cover them (the pool shares one completion semaphore).
            dma_eng = [nc.sync, nc.gpsimd, nc.scalar]
            nc.gpsimd.dma_start(out=biasT["q"][:], in_=bqT_d[:])
            nc.scalar.dma_start(out=biasT["k"][:], in_=bkT_d[:])
            nc.sync.dma_start(out=xT["q"][:], in_=xdr["q"][:])
            nc.gpsimd.dma_start(out=xT["k"][:], in_=xdr["k"][:])
            for di, (name, ch) in enumerate(
                    [(nm, ch) for ch in range(4) for nm in ("q", "k")]):
                dma_eng[(di + 2) % 3].dma_start(
                    out=Wsb[name][:, ch * 512:(ch + 1) * 512],
                    in_=wdr[name][:, ch * 512:(ch + 1) * 512],
                )

            # Q^T / K^T slabs in s' order: col (s, 16j + m); the eviction
            # scatters column m with stride 16 (matmul APs must be 2D, and
            # walrus crashes on 3D ones).  Slabs stay fp32r: the scatter
            # runs at the fast fp32 DVE rate, and fp32r matmuls with a
            # 512-wide moving operand are full speed.
            QKp = {}
            for name in ("q", "k"):
                dst = slab.tile([128, NSLICE * 2048], F32R, tag=f"{name}T")
                QKp[name] = dst
            for m in range(16):
                for name in ("q", "k"):
                    dst = QKp[name]
                    pt = psS.tile([128, 1024], F32, tag="sc")
                    nc.tensor.matmul(
                        pt[:, :512],
                        lhsT=Wsb[name][:, m * 128:(m + 1) * 128],
                        rhs=xT[name][:],
                        start=True, stop=True,
                    )
                    dview = dst[:].rearrange(
                        "p (s j w) -> p s j w", s=NSLICE, w=16)[:, :, :, m]
                    nc.vector.tensor_scalar(
                        dview,
                        pt[:, :512].rearrange("p (s j) -> p s j", s=NSLICE),
                        biasT[name][:, m:m + 1], None, ALU.add)

            # Phase-2 DMAs: V inputs + weights, issued while the Q/K
            # projection computes (V output isn't consumed until the
            # first AV matmul, well into the attention phase).
            nc.gpsimd.dma_start(out=ones_sb[:], in_=ones_d[:])
            nc.scalar.dma_start(out=bv_sb[:], in_=bv_d[:])
            nc.sync.dma_start(out=xT["v"][:], in_=xdr["v"][:])
            for ch in range(4):
                dma_eng[ch % 3].dma_start(
                    out=Wsb["v"][:, ch * 512:(ch + 1) * 512],
                    in_=wdr["v"][:, ch * 512:(ch + 1) * 512],
                )
            nc.sync.dma_start(out=wo4[:], in_=Wo4_d[:])

            # ---- projections for ALL 4 slices up-front ----
            # V: natural slab -> DRAM bounce -> chunk layout [k, (i, d)]
            vch = slab.tile([128, NSLICE * 2048], BF16, tag="vch")
            for sl in range(NSLICE):
                vsl = vslp.tile([128, 2048], BF16, tag="vsl")
                for qtr in range(4):
                    vq = psA.tile([128, 512], F32, tag="av")
                    nc.tensor.matmul(
                        vq[:],
                        lhsT=ones_sb[0:1, :],
                        rhs=bv_sb[0:1, qtr * 512:(qtr + 1) * 512],
                        start=True, stop=False,
                    )
                    nc.tensor.matmul(
                        vq[:],
                        lhsT=xT["v"][:, sl * 128:(sl + 1) * 128],
                        rhs=Wsb["v"][:, qtr * 512:(qtr + 1) * 512],
                        start=False, stop=True,
                    )
                    nc.scalar.activation(
                        vsl[:, qtr * 512:(qtr + 1) * 512], vq[:], AF.Copy)
                # vch[16u+w, (i,d)] = vsl[8i+u, 128w+d].  SBUF partition
                # dims can't be re-split by rearrange, but a DMA only needs
                # matching element order: dest [128,128] iterates (16u+w, d)
                # exactly as source [8,16,128] iterates (u, w, d).
                nc.sync.dma_start(out=vscr_d[sl], in_=vsl[:])
                nc.sync.dma_start(
                    out=vch[:, sl * 2048:(sl + 1) * 2048].rearrange(
                        "p (i d) -> p i d", i=16),
                    in_=vscr_d[sl].rearrange(
                        "(i u) (w d) -> (u w) i d", u=8, w=16),
                )

            # ---- attention: panel-outer (desc), slice-inner ----
            # The PE queue is in-order, so consumer matmuls are software-
            # pipelined behind their producers via a deferred-emission
            # queue: AV/lB for group g land two scores pairs later (hiding
            # the exp + causal-mask latency), and the recip/mult + wop +
            # panel eviction for a slice land after the first scores pairs
            # of the NEXT slice (hiding the DVE latency).  Pops per flush
            # point (2, 2, 1, 1, ...) balance the G+2 items each slice
            # appends, giving a steady queue of [avlb(G-2), avlb(G-1),
            # epiA, epiB] at slice boundaries.  PSUM tiles are allocated
            # lazily inside the deferred emitters so pool rotation only
            # ever reuses a tile whose readers are already emitted.
            QT_all = QKp["q"]
            KT_all = QKp["k"]
            pending = deque()   # deferred emitters

            def flush(n):
                for _ in range(min(n, len(pending))):
                    pending.popleft()()

            def mk_avlb(g, st, pbs, VC, nchunk):
                def go():
                    if g == 0:
                        st["oT"] = psA.tile([128, 512], F32, tag="av", name="oT")
                        st["lB"] = psA.tile([128, 512], F32, tag="av", name="lB")
                    oT, lB = st["oT"], st["lB"]
                    pb = pbs[g]
                    # group by stationary operand: both AV chunks, then
                    # both lB chunks (ones stays loaded on the PE)
                    for half in range(2):
                        i = 2 * g + half
                        nc.tensor.matmul(
                            oT[:],
                            lhsT=VC[:, i * 128:(i + 1) * 128],
                            rhs=pb[:, half * 512:(half + 1) * 512],
                            start=(i == 0), stop=(i == nchunk - 1),
                        )
                    for half in range(2):
                        i = 2 * g + half
                        nc.tensor.matmul(
                            lB[:],
                            lhsT=ones_sb[:],
                            rhs=pb[:, half * 512:(half + 1) * 512],
                            start=(i == 0), stop=(i == nchunk - 1),
                        )
                return go

            def mk_epiA(st):
                def go():
                    # normalize oT into bf16 osbn for the Wo fold
                    rb = rbp.tile([128, 512], F32, tag="rb")
                    nc.vector.reciprocal_approx_fast(
                        out=rb[:], in_=st["lB"][:])
                    osbn = osbp.tile([128, 512], BF16, tag="osbn")
                    nc.vector.tensor_tensor(
                        osbn[:], st["oT"][:], rb[:], ALU.mult)
                    st["osbn"] = osbn
                return go

            def mk_epiB(sl, p, st, pst):
                def go():
                    if sl == 0:
                        pst["acps"] = psO.tile([128, 512], F32, tag="acps", name="acps")
                    acps = pst["acps"]
                    nc.tensor.matmul(
                        acps[:],
                        lhsT=wo4[:, sl * 128:(sl + 1) * 128],
                        rhs=st["osbn"][:],
                        start=(sl == 0), stop=(sl == NSLICE - 1),
                    )
                    if sl == NSLICE - 1:
                        acc_sb = accp.tile([128, 512], BF16, tag="acc")
                        nc.vector.tensor_copy(acc_sb[:], acps[:])
                        nc.sync.dma_start(
                            out=out_d[:, p * 512:(p + 1) * 512],
                            in_=acc_sb[:])
                return go

            # unit order: long panels sequentially, then the two short
            # panels' slices interleaved (two independent chains keep the
            # PE busy through the pipeline drain of each short slice)
            units = []
            for sl in range(NSLICE):
                units += [(3, sl), (0, sl)]
            for sl in range(NSLICE):
                units += [(2, sl), (1, sl)]
            psts = {p: {} for p in range(NPANEL)}
            for p, sl in units:
                if True:
                    pst = psts[p]
                    nchunk = 4 * p + 4   # causal: k-chunks 0..4p+3
                    VC = vch[:, sl * 2048:(sl + 1) * 2048]
                    st = {}
                    pbs = {}
                    for g in range(nchunk // 2):
                        sc = psS.tile([128, 1024], F32, tag="sc")
                        for half in range(2):
                            i = 2 * g + half
                            nc.tensor.matmul(
                                sc[:, half * 512:(half + 1) * 512],
                                lhsT=KT_all[:, sl * 2048 + i * 128:
                                            sl * 2048 + (i + 1) * 128],
                                rhs=QT_all[:, sl * 2048 + p * 512:
                                           sl * 2048 + (p + 1) * 512],
                                start=True, stop=True,
                            )
                        flush(2 if g <= 1 else 1)
                        pb = pbp.tile([128, 1024], BF16, tag="pb")
                        pbs[g] = pb
                        nc.scalar.activation(pb[:], sc[:], AF.Exp, scale=SCALE)
                        r0 = 2 * g - 4 * p  # r for half 0; half adds 1
                        if r0 >= 0:
                            # zero where q < k: keep phi - 128*half - kappa
                            # - 128*r0 >= 0 (one select covers both halves)
                            nc.gpsimd.affine_select(
                                out=pb[:].rearrange("p (h f) -> p h f", h=2),
                                in_=pb[:].rearrange("p (h f) -> p h f", h=2),
                                compare_op=ALU.is_ge,
                                fill=0.0,
                                base=-128 * r0,
                                pattern=[[-128, 2], [1, 512]],
                                channel_multiplier=-1,
                            )
                        pending.append(mk_avlb(g, st, pbs, VC, nchunk))
                    pending.append(mk_epiA(st))
                    pending.append(mk_epiB(sl, p, st, pst))
            flush(len(pending))

    nc.compile()
    return nc


def kernel(query, key, values, Wq, bq, Wk, bk, Wv, bv, Wo, bo, mask):
    assert mask, "kernel compiled for causal attention (mask truthy)"
    query = np.asarray(query, np.float32)
    key = np.asarray(key, np.float32)
    values = np.asarray(values, np.float32)
    Wq_ = np.ascontiguousarray(np.asarray(Wq, np.float32)).astype(BF_NP)
    Wk_ = np.ascontiguousarray(np.asarray(Wk, np.float32)).astype(BF_NP)
    Wv_ = np.ascontiguousarray(np.asarray(Wv, np.float32)).astype(BF_NP)
    Wo_ = np.asarray(Wo, np.float32)
    bqT = np.ascontiguousarray(np.asarray(bq, np.float32).reshape(16, 128).T)
    bkT = np.ascontiguousarray(np.asarray(bk, np.float32).reshape(16, 128).T)
    bv_r = np.ascontiguousarray(
        np.asarray(bv, np.float32).reshape(1, 2048)).astype(BF_NP)

    if "nc" not in _CACHE:
        _CACHE["nc"] = _build()
    nc = _CACHE["nc"]

    in_maps = []
    for c in range(NCORES):
        b = c // 4
        heads = [4 * (c % 4) + t for t in range(NSLICE)]
        qT = np.concatenate(
            [query[b, 128 * h:128 * (h + 1), :].T for h in heads], axis=1)
        kT = np.concatenate(
            [key[b, 128 * h:128 * (h + 1), :].T for h in heads], axis=1)
        vT = np.concatenate(
            [values[b, 128 * h:128 * (h + 1), :].T for h in heads], axis=1)
        Wo4 = np.concatenate(
            [Wo_[128 * h:128 * (h + 1), :] for h in heads], axis=1)
        in_maps.append({
            "qT": np.ascontiguousarray(qT).astype(BF_NP),
            "kT": np.ascontiguousarray(kT).astype(BF_NP),
            "vT": np.ascontiguousarray(vT).astype(BF_NP),
            "Wq": Wq_, "Wk": Wk_, "Wv": Wv_,
            "Wo4": np.ascontiguousarray(Wo4).astype(BF_NP),
            "bqT": bqT, "bkT": bkT, "bv_r": bv_r,
            "ones": _ONES,
        })

    _CACHE["last_in_maps"] = in_maps
    res = run_bass_kernel_spmd(nc, in_maps, list(range(NCORES)))
    out = np.empty((B, S, D), np.float32)
    bo_ = np.asarray(bo, np.float32)
    for b in range(B):
        part = res.results[4 * b]["partial"].astype(np.float32)
        for i in range(1, 4):
            part += res.results[4 * b + i]["partial"].astype(np.float32)
        out[b] = part.T + bo_
    return out
